# revision 13
# baseline (speedup 1.0000x reference)
"""CIN kernel: per-block L1/L2 pipeline + packed fp8 L0 + deferred scores."""

import numpy as np
import ml_dtypes
from contextlib import ExitStack

import concourse.bass as bass
import concourse.tile as tile
from concourse import bacc, mybir
from concourse.bass_utils import run_bass_kernel_spmd

F0 = 39
D = 16
B = 8192
NCORES = 8
BC = B // NCORES
N = BC * D
CH = 1024
NCHUNK = N // CH
NBLK = 20
NP0 = 6
FP16 = mybir.dt.float16
FP32 = mybir.dt.float32
FP8 = mybir.dt.float8e4

_BUILT = None


def _build_program():
    nc = bacc.Bacc(
        "TRN2",
        target_bir_lowering=False,
        debug=False,
        num_devices=NCORES,
    )

    x2_d = nc.dram_tensor("x2", [40, N], FP16, kind="ExternalInput").ap()
    xi_d = nc.dram_tensor("xi", [NP0 * 128, N], FP16, kind="ExternalInput").ap()
    xj_d = nc.dram_tensor("xj", [NP0 * 128, N], FP16, kind="ExternalInput").ap()
    f0_d = nc.dram_tensor("f0", [128, NP0 * 128], FP8, kind="ExternalInput").ap()
    f_d = [
        nc.dram_tensor(f"f{k}", [128, NBLK * 128], FP16, kind="ExternalInput").ap()
        for k in (1, 2)
    ]
    f8_d = [
        nc.dram_tensor(f"f{k}b8", [128, 2 * 128], FP8, kind="ExternalInput").ap()
        for k in (1, 2)
    ]
    wv_d = nc.dram_tensor("wv", [128, 3], FP16, kind="ExternalInput").ap()
    bias_d = nc.dram_tensor("bias", [1, 1], FP32, kind="ExternalInput").ap()
    out_d = nc.dram_tensor("out", [1, BC], FP32, kind="ExternalOutput").ap()

    relu = mybir.ActivationFunctionType.Relu

    with tile.TileContext(nc) as tc, ExitStack() as ctx:
        const = ctx.enter_context(tc.tile_pool(name="const", bufs=1))
        pool_a = ctx.enter_context(tc.tile_pool(name="a", bufs=2))
        pool_x = ctx.enter_context(tc.tile_pool(name="x", bufs=2))
        pool_z = ctx.enter_context(tc.tile_pool(name="z", bufs=8))
        pool_h = ctx.enter_context(tc.tile_pool(name="h", bufs=3))
        pool_dt = ctx.enter_context(tc.tile_pool(name="dt", bufs=5))
        pool_r2 = ctx.enter_context(tc.tile_pool(name="r2", bufs=3))
        pool_ah = ctx.enter_context(tc.tile_pool(name="ah", bufs=2))
        pool_z8 = ctx.enter_context(tc.tile_pool(name="z8", bufs=6))
        ps_curr = ctx.enter_context(tc.tile_pool(name="pcur", bufs=3, space="PSUM"))
        ps_s = ctx.enter_context(tc.tile_pool(name="ps", bufs=2, space="PSUM"))

        f0sb = const.tile([128, NP0 * 128], FP8, tag="f0sb")
        nc.sync.dma_start(f0sb[:], f0_d[:])
        fsb = []
        for k in range(2):
            f = const.tile([128, NBLK * 128], FP16, tag=f"f{k}", name=f"fc{k}")
            nc.gpsimd.dma_start(f[:, : NBLK * 64], f_d[k][:, : NBLK * 64])
            nc.gpsimd.dma_start(f[:, NBLK * 64 :], f_d[k][:, NBLK * 64 :])
            fsb.append(f)
        f8sb = []
        for k in range(2):
            f8 = const.tile([128, 2 * 128], FP8, tag=f"f8{k}", name=f"f8c{k}")
            nc.sync.dma_start(f8[:], f8_d[k][:])
            f8sb.append(f8)
        wv = const.tile([128, 3], FP16)
        nc.sync.dma_start(wv[:], wv_d[:])
        bias = const.tile([1, 1], FP32)
        nc.sync.dma_start(bias[:], bias_d[:])
        scores = const.tile([1, BC], FP32)

        def load_chunk(c):
            sl = slice(c * CH, (c + 1) * CH)
            xia = pool_x.tile([128, NP0 * CH], FP16, tag="xia", name=f"xia_{c}")
            xja = pool_x.tile([128, NP0 * CH], FP16, tag="xja", name=f"xja_{c}")
            src_i = xi_d[:, sl].rearrange("(b p) c -> p b c", p=128)
            src_j = xj_d[:, sl].rearrange("(b p) c -> p b c", p=128)
            nc.gpsimd.dma_start(xia[:].rearrange("p (b c) -> p b c", b=NP0), src_i)
            nc.gpsimd.dma_start(xja[:].rearrange("p (b c) -> p b c", b=NP0), src_j)
            return xia, xja

        def load_a_half(c, half):
            sl = slice(c * CH, (c + 1) * CH)
            pool = pool_a if half == 0 else pool_ah
            a_t = pool.tile([128, 10 * CH], FP16, tag="a", name=f"a_{c}_{half}")
            rows = x2_d[20 * half : 20 * half + 20, sl].rearrange(
                "(j two) c -> two j c", two=2
            )
            for a in range(2):
                nc.gpsimd.dma_start(
                    a_t[64 * a : 64 * (a + 1)].rearrange("p (j c) -> p j c", j=10),
                    rows[a : a + 1].to_broadcast([64, 10, CH]),
                )
            return a_t

        def layer_pass(c, layer, a_lo, a_hi, b_op, fw, f8w):
            cur = ps_curr.tile([128, CH], FP32, tag="cur", name=f"cur_{c}_{layer}")
            z8 = pool_z8.tile([128, 2 * CH], FP8, tag="z8", name=f"z8l_{c}_{layer}")
            nc.gpsimd.tensor_tensor(
                out=z8[:].rearrange("p (b c) -> p b c", b=2),
                in0=a_hi[:, 8 * CH :].rearrange("p (b c) -> p b c", b=2),
                in1=b_op[:, None, :].to_broadcast([128, 2, CH]),
                op=mybir.AluOpType.mult,
            )
            for k in range(18):
                at = a_lo if k < 10 else a_hi
                ko = k if k < 10 else k - 10
                z = pool_z.tile([128, CH], FP16, tag="z", name=f"z_{c}_{layer}_{k}")
                nc.vector.tensor_tensor(
                    out=z[:], in0=at[:, ko * CH : (ko + 1) * CH], in1=b_op[:],
                    op=mybir.AluOpType.mult,
                )
                for sgn in range(2):
                    ssl = slice(sgn * 512, (sgn + 1) * 512)
                    nc.tensor.matmul(
                        cur[:, ssl],
                        lhsT=fw[:, k * 128 : (k + 1) * 128],
                        rhs=z[:, ssl],
                        start=(k == 0),
                        stop=False,
                    )
            zr = z8[:].rearrange("p (two c) -> p two c", two=2)
            lw = f8w[:].rearrange("p (two m) -> p two m", two=2)
            for sgn in range(2):
                nc.tensor.matmul(
                    cur[:, sgn * 512 : (sgn + 1) * 512],
                    lhsT=lw,
                    rhs=zr[:, :, sgn * 512 : (sgn + 1) * 512],
                    start=False, stop=True,
                    perf_mode=mybir.MatmulPerfMode.DoubleRow,
                )
            return cur

        def score_mms(sab, layer, rhs_t, rhs_k):
            for sgn in range(2):
                ssl = slice(sgn * 512, (sgn + 1) * 512)
                nc.tensor.matmul(
                    sab[32 * sgn : 32 * sgn + 1, :],
                    lhsT=wv[0:rhs_k, layer : layer + 1],
                    rhs=rhs_t[0:rhs_k, ssl],
                    start=(layer == 0), stop=(layer == 2),
                    tile_position=(0, 32 * sgn),
                )

        def do_l0(c, xia, xja):
            cur = ps_curr.tile([128, CH], FP32, tag="cur", name=f"cur0_{c}")
            z8s = []
            for q in range(3):
                z8 = pool_z8.tile([128, 2 * CH], FP8, tag="z8", name=f"z80_{c}_{q}")
                sl2 = slice(2 * q * CH, (2 * q + 2) * CH)
                nc.gpsimd.tensor_tensor(
                    out=z8[:], in0=xia[:, sl2], in1=xja[:, sl2],
                    op=mybir.AluOpType.mult,
                )
                z8s.append(z8)
            for q in range(3):
                lw = f0sb[:, q * 256 : (q + 1) * 256].rearrange(
                    "p (two m) -> p two m", two=2
                )
                zr = z8s[q][:].rearrange("p (two c) -> p two c", two=2)
                for sgn in range(2):
                    nc.tensor.matmul(
                        cur[:, sgn * 512 : (sgn + 1) * 512],
                        lhsT=lw,
                        rhs=zr[:, :, sgn * 512 : (sgn + 1) * 512],
                        start=(q == 0), stop=(q == 2),
                        perf_mode=mybir.MatmulPerfMode.DoubleRow,
                    )
            h_t = pool_h.tile([128, CH], FP16, tag="h", name=f"h_{c}")
            d_t = pool_dt.tile([64, CH], FP16, tag="d", name=f"d_{c}")
            nc.scalar.activation(h_t[0:64, :], cur[0:64, :], relu, scale=1.0 / 64)
            nc.scalar.activation(h_t[64:128, :], cur[0:64, :], relu, scale=1.0 / 64)
            nc.scalar.activation(d_t[:], cur[64:128, :], relu, scale=1.0 / 64)
            return h_t, d_t

        chunks = {}
        for cc in range(2):
            xv = load_chunk(cc)
            chunks[cc] = (load_a_half(cc, 0), load_a_half(cc, 1), xv[0], xv[1])
        state = do_l0(0, chunks[0][2], chunks[0][3])

        def emit_reduces(t, sab_t):
            for sgn in range(2):
                off = t * (CH // D) + sgn * 32
                nc.vector.tensor_reduce(
                    out=scores[0:1, off : off + 32],
                    in_=sab_t[32 * sgn : 32 * sgn + 1, :].rearrange(
                        "p (g x) -> p g x", x=D
                    ),
                    axis=mybir.AxisListType.X,
                    op=mybir.AluOpType.add,
                )

        pending_reduce = None
        for t in range(NCHUNK):
            a_lo, a_hi = chunks[t][0], chunks[t][1]
            h1, d_t = state
            cur1 = layer_pass(t, 1, a_lo, a_hi, h1, fsb[0], f8sb[0])
            if pending_reduce is not None:
                pt, pd0, pd1, pr2 = pending_reduce
                psab = ps_s.tile([33, 512], FP32, tag="sab", name=f"sab_{pt}")
                score_mms(psab, 0, pd0, 64)
                score_mms(psab, 1, pd1, 64)
                score_mms(psab, 2, pr2, 128)
                emit_reduces(pt, psab)
            h2 = pool_h.tile([128, CH], FP16, tag="h", name=f"h2_{t}")
            d1 = pool_dt.tile([64, CH], FP16, tag="d", name=f"d1_{t}")
            nc.scalar.activation(h2[0:64, :], cur1[0:64, :], relu, scale=1.0 / 64)
            nc.scalar.activation(h2[64:128, :], cur1[0:64, :], relu, scale=1.0 / 64)
            nc.scalar.activation(d1[:], cur1[64:128, :], relu, scale=1.0 / 64)
            if t + 1 < NCHUNK:
                state = do_l0(t + 1, chunks[t + 1][2], chunks[t + 1][3])
            al2 = load_a_half(t + 2, 0) if t + 2 < NCHUNK else None
            cur2 = layer_pass(t, 2, a_lo, a_hi, h2, fsb[1], f8sb[1])
            r2 = pool_r2.tile([128, CH], FP16, tag="r2", name=f"r2_{t}")
            nc.scalar.activation(r2[:], cur2[:], relu, scale=1.0 / 64)
            if t + 2 < NCHUNK:
                xv = load_chunk(t + 2)
                chunks[t + 2] = (al2, load_a_half(t + 2, 1), xv[0], xv[1])
            del chunks[t]
            pending_reduce = (t, d_t, d1, r2)

        pt, pd0, pd1, pr2 = pending_reduce
        psab = ps_s.tile([33, 512], FP32, tag="sab", name=f"sab_{pt}")
        score_mms(psab, 0, pd0, 64)
        score_mms(psab, 1, pd1, 64)
        score_mms(psab, 2, pr2, 128)
        emit_reduces(pt, psab)
        nc.vector.tensor_scalar_add(scores[:], scores[:], bias[0:1, 0:1])
        nc.sync.dma_start(out_d[:], scores[:])

    nc.compile()
    return nc


def _prep_inputs(nn_input, f0, f1, f2, w_nn, b_nn):
    nn_input = np.asarray(nn_input, dtype=np.float32)
    f0 = np.asarray(f0, dtype=np.float32)
    f1 = np.asarray(f1, dtype=np.float32)
    f2 = np.asarray(f2, dtype=np.float32)
    w_nn = np.asarray(w_nn, dtype=np.float32).reshape(-1)
    b_nn = np.asarray(b_nn, dtype=np.float32).reshape(-1)

    def pack(fp):
        blocks = fp.reshape(NBLK, 128, 128)
        return np.ascontiguousarray(
            blocks.transpose(1, 0, 2).reshape(128, NBLK * 128)
        ).astype(np.float16)

    def padf(f):
        out = np.zeros((2560, 128), np.float32)
        out[: F0 * 64] = 64.0 * f
        blocks = out.reshape(NBLK, 128, 128).transpose(1, 0, 2)
        f16p = np.ascontiguousarray(blocks.reshape(128, NBLK * 128)).astype(
            np.float16
        )
        f8p = np.ascontiguousarray(blocks[:, 18:].reshape(128, 2 * 128)).astype(
            ml_dtypes.float8_e4m3
        )
        return f16p, f8p

    f1p, f1b8 = padf(f1)
    f2p, f2b8 = padf(f2)

    iu, ju = np.triu_indices(F0, k=1)
    f0r = f0.reshape(F0, F0, 128)
    w0 = np.zeros((NP0 * 128, 128), np.float32)
    w0[: len(iu)] = 2.0 * 64.0 * f0r[iu, ju]
    w0b = w0.reshape(NP0, 128, 128).transpose(1, 0, 2).reshape(128, NP0 * 128)
    f0p = np.ascontiguousarray(w0b).astype(ml_dtypes.float8_e4m3)

    wv = np.zeros((128, 3), np.float32)
    wv[0:64, 0] = 1.0 + w_nn[0:64]
    wv[0:64, 1] = 1.0 + w_nn[64:128]
    wv[:, 2] = 1.0 + w_nn[128:256]
    wv = wv.astype(np.float16)
    bias = b_nn.reshape(1, 1).astype(np.float32)

    x0 = nn_input.reshape(B, F0, D)
    in_maps = []
    for cidx in range(NCORES):
        xc = x0[cidx * BC : (cidx + 1) * BC]
        xt = xc.transpose(1, 0, 2).reshape(F0, N).astype(np.float16)
        x2h = np.zeros((40, N), np.float16)
        x2h[:F0] = xt
        xi = np.zeros((NP0 * 128, N), np.float16)
        xj = np.zeros((NP0 * 128, N), np.float16)
        xi[: len(iu)] = xt[iu]
        xj[: len(ju)] = xt[ju]
        in_maps.append(
            {"x2": x2h, "xi": xi, "xj": xj, "f0": f0p,
             "f1": f1p, "f2": f2p, "f1b8": f1b8, "f2b8": f2b8,
             "wv": wv, "bias": bias}
        )
    return in_maps


def _run(inputs, trace=False, trace_kwargs=None):
    global _BUILT
    if _BUILT is None:
        _BUILT = _build_program()
    nc = _BUILT
    in_maps = _prep_inputs(**inputs)
    res = run_bass_kernel_spmd(
        nc,
        in_maps,
        core_ids=list(range(NCORES)),
        trace=trace,
        **(trace_kwargs or {}),
    )
    out = np.concatenate(
        [res.results[c]["out"].reshape(BC) for c in range(NCORES)]
    )
    return out.reshape(B, 1).astype(np.float32), res


def kernel(**inputs):
    out, _ = _run(inputs)
    return out


# revision 15
# speedup vs baseline: 1.0124x; 1.0124x over previous
"""CIN kernel: per-block L1/L2 pipeline + packed fp8 L0 + deferred scores."""

import numpy as np
import ml_dtypes
from contextlib import ExitStack

import concourse.bass as bass
import concourse.tile as tile
from concourse import bacc, mybir
from concourse.bass_utils import run_bass_kernel_spmd

F0 = 39
D = 16
B = 8192
NCORES = 8
BC = B // NCORES
N = BC * D
CH = 1024
NCHUNK = N // CH
NBLK = 20
NP0 = 6
FP16 = mybir.dt.float16
FP32 = mybir.dt.float32
FP8 = mybir.dt.float8e4

_BUILT = None


def _build_program():
    nc = bacc.Bacc(
        "TRN2",
        target_bir_lowering=False,
        debug=False,
        num_devices=NCORES,
    )

    x2_d = nc.dram_tensor("x2", [40, N], FP16, kind="ExternalInput").ap()
    xi_d = nc.dram_tensor("xi", [NP0 * 128, N], FP16, kind="ExternalInput").ap()
    xj_d = nc.dram_tensor("xj", [NP0 * 128, N], FP16, kind="ExternalInput").ap()
    f0_d = nc.dram_tensor("f0", [128, NP0 * 128], FP8, kind="ExternalInput").ap()
    f_d = [
        nc.dram_tensor(f"f{k}", [128, NBLK * 128], FP16, kind="ExternalInput").ap()
        for k in (1, 2)
    ]
    f8_d = [
        nc.dram_tensor(f"f{k}b8", [128, 2 * 128], FP8, kind="ExternalInput").ap()
        for k in (1, 2)
    ]
    wv_d = nc.dram_tensor("wv", [128, 3], FP16, kind="ExternalInput").ap()
    bias_d = nc.dram_tensor("bias", [1, 1], FP32, kind="ExternalInput").ap()
    out_d = nc.dram_tensor("out", [1, BC], FP32, kind="ExternalOutput").ap()

    relu = mybir.ActivationFunctionType.Relu

    with tile.TileContext(nc) as tc, ExitStack() as ctx:
        const = ctx.enter_context(tc.tile_pool(name="const", bufs=1))
        pool_a = ctx.enter_context(tc.tile_pool(name="a", bufs=2))
        pool_x = ctx.enter_context(tc.tile_pool(name="x", bufs=2))
        pool_z = ctx.enter_context(tc.tile_pool(name="z", bufs=8))
        pool_h = ctx.enter_context(tc.tile_pool(name="h", bufs=3))
        pool_dt = ctx.enter_context(tc.tile_pool(name="dt", bufs=5))
        pool_r2 = ctx.enter_context(tc.tile_pool(name="r2", bufs=3))
        pool_ah = ctx.enter_context(tc.tile_pool(name="ah", bufs=2))
        pool_z8 = ctx.enter_context(tc.tile_pool(name="z8", bufs=6))
        ps_curr = ctx.enter_context(tc.tile_pool(name="pcur", bufs=3, space="PSUM"))
        ps_s = ctx.enter_context(tc.tile_pool(name="ps", bufs=2, space="PSUM"))

        f0sb = const.tile([128, NP0 * 128], FP8, tag="f0sb")
        nc.sync.dma_start(f0sb[:], f0_d[:])
        fsb = []
        for k in range(2):
            f = const.tile([128, NBLK * 128], FP16, tag=f"f{k}", name=f"fc{k}")
            nc.gpsimd.dma_start(f[:, : NBLK * 64], f_d[k][:, : NBLK * 64])
            nc.gpsimd.dma_start(f[:, NBLK * 64 :], f_d[k][:, NBLK * 64 :])
            fsb.append(f)
        f8sb = []
        for k in range(2):
            f8 = const.tile([128, 2 * 128], FP8, tag=f"f8{k}", name=f"f8c{k}")
            nc.sync.dma_start(f8[:], f8_d[k][:])
            f8sb.append(f8)
        wv = const.tile([128, 3], FP16)
        nc.sync.dma_start(wv[:], wv_d[:])
        bias = const.tile([1, 1], FP32)
        nc.sync.dma_start(bias[:], bias_d[:])
        scores = const.tile([1, BC], FP32)

        def load_chunk(c):
            sl = slice(c * CH, (c + 1) * CH)
            xia = pool_x.tile([128, NP0 * CH], FP16, tag="xia", name=f"xia_{c}")
            xja = pool_x.tile([128, NP0 * CH], FP16, tag="xja", name=f"xja_{c}")
            src_i = xi_d[:, sl].rearrange("(b p) c -> p b c", p=128)
            src_j = xj_d[:, sl].rearrange("(b p) c -> p b c", p=128)
            nc.gpsimd.dma_start(xia[:].rearrange("p (b c) -> p b c", b=NP0), src_i)
            nc.gpsimd.dma_start(xja[:].rearrange("p (b c) -> p b c", b=NP0), src_j)
            return xia, xja

        def load_a_half(c, half):
            sl = slice(c * CH, (c + 1) * CH)
            pool = pool_a if half == 0 else pool_ah
            a_t = pool.tile([128, 10 * CH], FP16, tag="a", name=f"a_{c}_{half}")
            rows = x2_d[20 * half : 20 * half + 20, sl].rearrange(
                "(j two) c -> two j c", two=2
            )
            for a in range(2):
                nc.gpsimd.dma_start(
                    a_t[64 * a : 64 * (a + 1)].rearrange("p (j c) -> p j c", j=10),
                    rows[a : a + 1].to_broadcast([64, 10, CH]),
                )
            return a_t

        def layer_pass(c, layer, a_lo, a_hi, b_op, fw, f8w):
            cur = ps_curr.tile([128, CH], FP32, tag="cur", name=f"cur_{c}_{layer}")
            z8 = pool_z8.tile([128, 2 * CH], FP8, tag="z8", name=f"z8l_{c}_{layer}")
            nc.gpsimd.tensor_tensor(
                out=z8[:].rearrange("p (b c) -> p b c", b=2),
                in0=a_hi[:, 8 * CH :].rearrange("p (b c) -> p b c", b=2),
                in1=b_op[:, None, :].to_broadcast([128, 2, CH]),
                op=mybir.AluOpType.mult,
            )
            for k in range(18):
                at = a_lo if k < 10 else a_hi
                ko = k if k < 10 else k - 10
                z = pool_z.tile([128, CH], FP16, tag="z", name=f"z_{c}_{layer}_{k}")
                nc.vector.tensor_tensor(
                    out=z[:], in0=at[:, ko * CH : (ko + 1) * CH], in1=b_op[:],
                    op=mybir.AluOpType.mult,
                )
                for sgn in range(2):
                    ssl = slice(sgn * 512, (sgn + 1) * 512)
                    nc.tensor.matmul(
                        cur[:, ssl],
                        lhsT=fw[:, k * 128 : (k + 1) * 128],
                        rhs=z[:, ssl],
                        start=(k == 0),
                        stop=False,
                    )
            zr = z8[:].rearrange("p (two c) -> p two c", two=2)
            lw = f8w[:].rearrange("p (two m) -> p two m", two=2)
            for sgn in range(2):
                nc.tensor.matmul(
                    cur[:, sgn * 512 : (sgn + 1) * 512],
                    lhsT=lw,
                    rhs=zr[:, :, sgn * 512 : (sgn + 1) * 512],
                    start=False, stop=True,
                    perf_mode=mybir.MatmulPerfMode.DoubleRow,
                )
            return cur

        def score_mms(sab, layer, rhs_t, rhs_k):
            for sgn in range(2):
                ssl = slice(sgn * 512, (sgn + 1) * 512)
                nc.tensor.matmul(
                    sab[32 * sgn : 32 * sgn + 1, :],
                    lhsT=wv[0:rhs_k, layer : layer + 1],
                    rhs=rhs_t[0:rhs_k, ssl],
                    start=(layer == 0), stop=(layer == 2),
                    tile_position=(0, 32 * sgn),
                )

        def do_l0_pool(c, xia, xja):
            """Pool's two z8 pairs, enqueued at iteration top for max lead."""
            out = []
            for q in (0, 2):
                z8 = pool_z8.tile([128, 2 * CH], FP8, tag="z8", name=f"z80_{c}_{q}")
                sl2 = slice(2 * q * CH, (2 * q + 2) * CH)
                nc.gpsimd.tensor_tensor(
                    out=z8[:], in0=xia[:, sl2], in1=xja[:, sl2],
                    op=mybir.AluOpType.mult,
                )
                out.append(z8)
            return out

        def do_l0(c, xia, xja, z8p):
            cur = ps_curr.tile([128, CH], FP32, tag="cur", name=f"cur0_{c}")
            z8m = pool_z8.tile([128, 2 * CH], FP8, tag="z8", name=f"z80_{c}_1")
            # the middle pair rides the DVE in its h2-relu wait window
            nc.vector.tensor_tensor(
                out=z8m[:], in0=xia[:, 2 * CH : 4 * CH], in1=xja[:, 2 * CH : 4 * CH],
                op=mybir.AluOpType.mult,
            )
            z8s = [z8p[0], z8m, z8p[1]]
            for q in range(3):
                lw = f0sb[:, q * 256 : (q + 1) * 256].rearrange(
                    "p (two m) -> p two m", two=2
                )
                zr = z8s[q][:].rearrange("p (two c) -> p two c", two=2)
                for sgn in range(2):
                    nc.tensor.matmul(
                        cur[:, sgn * 512 : (sgn + 1) * 512],
                        lhsT=lw,
                        rhs=zr[:, :, sgn * 512 : (sgn + 1) * 512],
                        start=(q == 0), stop=(q == 2),
                        perf_mode=mybir.MatmulPerfMode.DoubleRow,
                    )
            h_t = pool_h.tile([128, CH], FP16, tag="h", name=f"h_{c}")
            d_t = pool_dt.tile([64, CH], FP16, tag="d", name=f"d_{c}")
            nc.scalar.activation(h_t[0:64, :], cur[0:64, :], relu, scale=1.0 / 64)
            nc.scalar.activation(h_t[64:128, :], cur[0:64, :], relu, scale=1.0 / 64)
            nc.scalar.activation(d_t[:], cur[64:128, :], relu, scale=1.0 / 64)
            return h_t, d_t

        chunks = {}
        for cc in range(2):
            xv = load_chunk(cc)
            chunks[cc] = (load_a_half(cc, 0), load_a_half(cc, 1), xv[0], xv[1])
        state = do_l0(0, chunks[0][2], chunks[0][3])

        def emit_reduces(t, sab_t):
            for sgn in range(2):
                off = t * (CH // D) + sgn * 32
                nc.vector.tensor_reduce(
                    out=scores[0:1, off : off + 32],
                    in_=sab_t[32 * sgn : 32 * sgn + 1, :].rearrange(
                        "p (g x) -> p g x", x=D
                    ),
                    axis=mybir.AxisListType.X,
                    op=mybir.AluOpType.add,
                )

        pending_reduce = None
        for t in range(NCHUNK):
            a_lo, a_hi = chunks[t][0], chunks[t][1]
            h1, d_t = state
            cur1 = layer_pass(t, 1, a_lo, a_hi, h1, fsb[0], f8sb[0])
            if pending_reduce is not None:
                pt, pd0, pd1, pr2 = pending_reduce
                psab = ps_s.tile([33, 512], FP32, tag="sab", name=f"sab_{pt}")
                score_mms(psab, 0, pd0, 64)
                score_mms(psab, 1, pd1, 64)
                score_mms(psab, 2, pr2, 128)
                emit_reduces(pt, psab)
            h2 = pool_h.tile([128, CH], FP16, tag="h", name=f"h2_{t}")
            d1 = pool_dt.tile([64, CH], FP16, tag="d", name=f"d1_{t}")
            nc.scalar.activation(h2[0:64, :], cur1[0:64, :], relu, scale=1.0 / 64)
            nc.scalar.activation(h2[64:128, :], cur1[0:64, :], relu, scale=1.0 / 64)
            nc.scalar.activation(d1[:], cur1[64:128, :], relu, scale=1.0 / 64)
            if t + 1 < NCHUNK:
                state = do_l0(t + 1, chunks[t + 1][2], chunks[t + 1][3])
            al2 = load_a_half(t + 2, 0) if t + 2 < NCHUNK else None
            cur2 = layer_pass(t, 2, a_lo, a_hi, h2, fsb[1], f8sb[1])
            r2 = pool_r2.tile([128, CH], FP16, tag="r2", name=f"r2_{t}")
            nc.scalar.activation(r2[:], cur2[:], relu, scale=1.0 / 64)
            if t + 2 < NCHUNK:
                xv = load_chunk(t + 2)
                chunks[t + 2] = (al2, load_a_half(t + 2, 1), xv[0], xv[1])
            del chunks[t]
            pending_reduce = (t, d_t, d1, r2)

        pt, pd0, pd1, pr2 = pending_reduce
        psab = ps_s.tile([33, 512], FP32, tag="sab", name=f"sab_{pt}")
        score_mms(psab, 0, pd0, 64)
        score_mms(psab, 1, pd1, 64)
        score_mms(psab, 2, pr2, 128)
        emit_reduces(pt, psab)
        nc.vector.tensor_scalar_add(scores[:], scores[:], bias[0:1, 0:1])
        nc.sync.dma_start(out_d[:], scores[:])

    nc.compile()
    return nc


def _prep_inputs(nn_input, f0, f1, f2, w_nn, b_nn):
    nn_input = np.asarray(nn_input, dtype=np.float32)
    f0 = np.asarray(f0, dtype=np.float32)
    f1 = np.asarray(f1, dtype=np.float32)
    f2 = np.asarray(f2, dtype=np.float32)
    w_nn = np.asarray(w_nn, dtype=np.float32).reshape(-1)
    b_nn = np.asarray(b_nn, dtype=np.float32).reshape(-1)

    def pack(fp):
        blocks = fp.reshape(NBLK, 128, 128)
        return np.ascontiguousarray(
            blocks.transpose(1, 0, 2).reshape(128, NBLK * 128)
        ).astype(np.float16)

    def padf(f):
        out = np.zeros((2560, 128), np.float32)
        out[: F0 * 64] = 64.0 * f
        blocks = out.reshape(NBLK, 128, 128).transpose(1, 0, 2)
        f16p = np.ascontiguousarray(blocks.reshape(128, NBLK * 128)).astype(
            np.float16
        )
        f8p = np.ascontiguousarray(blocks[:, 18:].reshape(128, 2 * 128)).astype(
            ml_dtypes.float8_e4m3
        )
        return f16p, f8p

    f1p, f1b8 = padf(f1)
    f2p, f2b8 = padf(f2)

    iu, ju = np.triu_indices(F0, k=1)
    f0r = f0.reshape(F0, F0, 128)
    w0 = np.zeros((NP0 * 128, 128), np.float32)
    w0[: len(iu)] = 2.0 * 64.0 * f0r[iu, ju]
    w0b = w0.reshape(NP0, 128, 128).transpose(1, 0, 2).reshape(128, NP0 * 128)
    f0p = np.ascontiguousarray(w0b).astype(ml_dtypes.float8_e4m3)

    wv = np.zeros((128, 3), np.float32)
    wv[0:64, 0] = 1.0 + w_nn[0:64]
    wv[0:64, 1] = 1.0 + w_nn[64:128]
    wv[:, 2] = 1.0 + w_nn[128:256]
    wv = wv.astype(np.float16)
    bias = b_nn.reshape(1, 1).astype(np.float32)

    x0 = nn_input.reshape(B, F0, D)
    in_maps = []
    for cidx in range(NCORES):
        xc = x0[cidx * BC : (cidx + 1) * BC]
        xt = xc.transpose(1, 0, 2).reshape(F0, N).astype(np.float16)
        x2h = np.zeros((40, N), np.float16)
        x2h[:F0] = xt
        xi = np.zeros((NP0 * 128, N), np.float16)
        xj = np.zeros((NP0 * 128, N), np.float16)
        xi[: len(iu)] = xt[iu]
        xj[: len(ju)] = xt[ju]
        in_maps.append(
            {"x2": x2h, "xi": xi, "xj": xj, "f0": f0p,
             "f1": f1p, "f2": f2p, "f1b8": f1b8, "f2b8": f2b8,
             "wv": wv, "bias": bias}
        )
    return in_maps


def _run(inputs, trace=False, trace_kwargs=None):
    global _BUILT
    if _BUILT is None:
        _BUILT = _build_program()
    nc = _BUILT
    in_maps = _prep_inputs(**inputs)
    res = run_bass_kernel_spmd(
        nc,
        in_maps,
        core_ids=list(range(NCORES)),
        trace=trace,
        **(trace_kwargs or {}),
    )
    out = np.concatenate(
        [res.results[c]["out"].reshape(BC) for c in range(NCORES)]
    )
    return out.reshape(B, 1).astype(np.float32), res


def kernel(**inputs):
    out, _ = _run(inputs)
    return out
